# revision 7
# baseline (speedup 1.0000x reference)
"""Trainium2 Bass kernel for nn_BiLSTMDualPathway_40596030881793.

Dual-pathway BiLSTM tagger: char BiLSTM (T=512, 2 layers, bidir) + word
BiLSTM (T=96), ragged word->char expansion, 2-layer classifier.

Sharding: pure data parallelism - batch 64 split 8 ways (8 samples/core),
weights replicated. On-device compute for everything except integer index
preprocessing (one-hot encodings) and weight layout / constant folding.

Device design:
- All matmuls bf16 -> fp32 PSUM.
- LSTM recurrence in gates^T [1024, 8] layout (gates on partitions, batch on
  free). Whh is the stationary operand: 16 [128,128] tiles per direction
  step. Precomputed input projections xp and biases are injected into PSUM
  by an identity-matmul and a bias-selector matmul, so ScalarE reads gate
  preactivations straight from PSUM.
- Gate tile order [i,f,o,g]: sigmoid covers a contiguous [128,48] slab,
  tanh a [128,16] slab.
- fwd/bwd directions interleave step-by-step so their PE burst -> ACT ->
  DVE cell-update chains dovetail across engines.
- Char xp lives in DRAM (16 MB/layer bf16), streamed in 8-step blocks;
  word xp stays in SBUF.
"""
import numpy as np
import ml_dtypes

B, L, W = 64, 512, 96
VOCAB, TAGS = 64, 15
CHAR_EMB, CHAR_H = 128, 256
WORD_EMB, WORD_H = 768, 256
NCORES = 8
BC = B // NCORES
CTOK = BC * L              # 4096
WTOK = BC * W              # 768
COMB = 2 * CHAR_H + 2 * WORD_H

bf16 = ml_dtypes.bfloat16

# gate permutation: pytorch [i,f,g,o] -> ours [i,f,o,g]
_PERM = np.concatenate([np.arange(0, 512), np.arange(768, 1024), np.arange(512, 768)])

_CACHE = {}


# ---------------------------------------------------------------- host prep
def _prep_weights(char_emb_table, char_lstm_params, word_lstm_params,
                  cls_W1, cls_b1, cls_W2, cls_b2):
    def lstm_mats(params):
        out = []
        for (pf, pb) in params:
            wih_cols, whh_blocks, biasT = [], [], []
            whhT = []
            for (Wih, Whh, bih, bhh) in (pf, pb):
                Wih = np.asarray(Wih, np.float32)[_PERM]
                Whh = np.asarray(Whh, np.float32)[_PERM]
                bias = (np.asarray(bih, np.float32) + np.asarray(bhh, np.float32))[_PERM]
                wih_cols.append(Wih.T)                  # [I, 1024]
                whhT.append(Whh.T)                      # [256, 1024]
                biasT.append(bias.reshape(8, 128))
            for d in range(2):
                for k in range(2):
                    for m in range(8):
                        whh_blocks.append(
                            whhT[d][k * 128:(k + 1) * 128, m * 128:(m + 1) * 128])
            out.append((np.concatenate(wih_cols, axis=1),       # [I, 2048]
                        np.concatenate(whh_blocks, axis=1),     # [128, 4096]
                        np.stack(biasT)))                       # [2, 8, 128]
        return out

    ch = lstm_mats(char_lstm_params)
    wd = lstm_mats(word_lstm_params)
    emb = np.asarray(char_emb_table, np.float32)
    return {
        "w0c": (emb @ ch[0][0]).astype(bf16),                   # [64, 2048]
        "wihc1": ch[1][0].astype(bf16),                         # [512, 2048]
        "whhc": np.stack([ch[0][1], ch[1][1]]).astype(bf16),    # [2, 128, 4096]
        "biasc": np.stack([ch[0][2], ch[1][2]]).astype(bf16),   # [2, 2, 8, 128]
        "wihw0": wd[0][0].astype(bf16),
        "wihw1": wd[1][0].astype(bf16),
        "whhw": np.stack([wd[0][1], wd[1][1]]).astype(bf16),
        "biasw": np.stack([wd[0][2], wd[1][2]]).astype(bf16),
        "w1": np.ascontiguousarray(np.asarray(cls_W1, np.float32).T).astype(bf16),
        "b1": np.ascontiguousarray(
            np.asarray(cls_b1, np.float32).reshape(4, 128).T).astype(np.float32),
        "w2": np.ascontiguousarray(np.asarray(cls_W2, np.float32).T).astype(bf16),
        "b2": np.asarray(cls_b2, np.float32).reshape(TAGS, 1).copy(),
        "ident": np.eye(128, dtype=np.float32).astype(bf16),
        "bsel": np.repeat(np.eye(8, dtype=np.float32), 8, axis=1).astype(bf16),
    }


def _prep_core_inputs(char_ids, word_embeddings, word_boundaries):
    ids = np.asarray(char_ids)                                  # [BC, L]
    oc = np.zeros((VOCAB, BC * L), np.float32)                  # tok = t*BC+b
    oc[ids.T.reshape(-1), np.arange(BC * L)] = 1.0

    we = np.asarray(word_embeddings, np.float32)                # [BC, 96, 768]
    wembT = np.transpose(we, (2, 1, 0)).reshape(WORD_EMB, WTOK)

    wb = np.asarray(word_boundaries, np.int64)
    cs = np.cumsum(wb, axis=1)
    pos = np.arange(L)
    ow = np.zeros((BC, W, L), np.float32)
    for b in range(BC):
        wid = np.searchsorted(cs[b], pos, side="right")
        valid = wid < W
        ow[b, wid[valid], pos[valid]] = 1.0
    ow = ow.transpose(1, 0, 2).reshape(W, BC * L)               # [96, (b,t)]
    return {
        "onehotc": oc.astype(bf16),
        "wembt": np.ascontiguousarray(wembT).astype(bf16),
        "onehotw": np.ascontiguousarray(ow).astype(bf16),
    }


# ---------------------------------------------------------------- program
def _build_program():
    import concourse.bacc as bacc
    import concourse.tile as tile
    import concourse.bass as bass
    from concourse import mybir
    from contextlib import ExitStack

    AF = mybir.ActivationFunctionType
    dtb = mybir.dt.bfloat16
    dtf = mybir.dt.float32
    MS = bass.MemorySpace

    nc = bacc.Bacc("TRN2", target_bir_lowering=False, debug=False,
                   num_devices=NCORES)

    def din(name, shape, dt=dtb):
        return nc.dram_tensor(name, shape, dt, kind="ExternalInput").ap()

    onehotc = din("onehotc", [VOCAB, CTOK])
    wembt = din("wembt", [WORD_EMB, WTOK])
    onehotw = din("onehotw", [W, CTOK])
    w0c = din("w0c", [VOCAB, 2048])
    wihc1 = din("wihc1", [512, 2048])
    whhc = din("whhc", [2, 128, 4096])
    biasc = din("biasc", [2, 2, 8, 128])
    wihw0 = din("wihw0", [WORD_EMB, 2048])
    wihw1 = din("wihw1", [512, 2048])
    whhw = din("whhw", [2, 128, 4096])
    biasw = din("biasw", [2, 2, 8, 128])
    w1d = din("w1", [COMB, 512])
    b1d = din("b1", [128, 4], dtf)
    w2d = din("w2", [512, TAGS])
    b2d = din("b2", [TAGS, 1], dtf)
    identd = din("ident", [128, 128])
    bseld = din("bsel", [8, 64])
    out = nc.dram_tensor("logits", [TAGS, CTOK], dtf, kind="ExternalOutput").ap()

    xpc = [nc.dram_tensor(f"xpc{l}", [2, 128, 8, CTOK], dtb).ap()
           for l in range(2)]

    with tile.TileContext(nc) as tc, ExitStack() as top:
        const = top.enter_context(tc.tile_pool(name="const", bufs=1))
        ident_sb = const.tile([128, 128], dtb)
        nc.sync.dma_start(ident_sb[:], identd[:])
        bsel_sb = const.tile([8, 64], dtb)
        nc.sync.dma_start(bsel_sb[:], bseld[:])

        longlive = top.enter_context(tc.tile_pool(name="longlive", bufs=1))
        hc1 = longlive.tile([128, (L + 2) * 32], dtb)
        wexp = longlive.tile([128, 4 * CTOK], dtb)

        # ----------------------------------------------------- projection
        def projection(name, lhsT_dram, K, rhs_fn, nchunk, chunk, evac_fn):
            """evac_fn(m, c, psum_ap); lhsT_dram [K, M*128]."""
            with ExitStack() as ctx:
                kt = (K + 127) // 128
                mt = lhsT_dram.shape[1] // 128
                pool = ctx.enter_context(tc.tile_pool(name=f"{name}w", bufs=2))
                pp = ctx.enter_context(
                    tc.tile_pool(name=f"{name}p", bufs=4, space=MS.PSUM))
                for m in range(mt):
                    wt = pool.tile([128, kt * 128], dtb, tag="w",
                                   name=f"{name}w{m}")
                    for k in range(kt):
                        kn = min(128, K - k * 128)
                        nc.sync.dma_start(
                            wt[:kn, k * 128:k * 128 + 128],
                            lhsT_dram[k * 128:k * 128 + kn,
                                      m * 128:(m + 1) * 128])
                    for c in range(nchunk):
                        ps = pp.tile([128, chunk], dtf, tag="ps",
                                     name=f"{name}ps")
                        for k in range(kt):
                            kn = min(128, K - k * 128)
                            nc.tensor.matmul(
                                ps[:], wt[:kn, k * 128:k * 128 + 128],
                                rhs_fn(k, kn, c),
                                start=(k == 0), stop=(k == kt - 1))
                        evac_fn(m, c, ps)

        # ----------------------------------------------------- recurrence
        def recurrence(name, T, whh_dram, bias_dram, layer, h_sb,
                       xp_sbuf=None, xp_dram=None):
            with ExitStack() as ctx:
                wp = ctx.enter_context(tc.tile_pool(name=f"{name}wp", bufs=1))
                whh_sb = wp.tile([128, 4096], dtb, name=f"{name}whh")
                nc.sync.dma_start(whh_sb[:], whh_dram[layer])
                bias_sb = wp.tile([8, 2 * 128], dtb, name=f"{name}bias")
                for d in range(2):
                    nc.sync.dma_start(bias_sb[:, d * 128:(d + 1) * 128],
                                      bias_dram[layer, d])
                c_sb = wp.tile([128, 2 * 2 * 16], dtf, name=f"{name}cs")
                nc.vector.memset(c_sb[:], 0.0)
                nc.vector.memset(h_sb[:, 0:32], 0.0)
                nc.vector.memset(h_sb[:, (T + 1) * 32:(T + 2) * 32], 0.0)
                xpool = ctx.enter_context(tc.tile_pool(name=f"{name}x", bufs=3))
                work = ctx.enter_context(tc.tile_pool(name=f"{name}k", bufs=3))
                pp = ctx.enter_context(
                    tc.tile_pool(name=f"{name}p", bufs=3, space=MS.PSUM))

                BLK = 8
                xt = {}
                for blk in range((T + BLK - 1) // BLK):
                    bT = min(BLK, T - blk * BLK)
                    if xp_dram is not None:
                        for d in range(2):
                            lo = blk * BLK if d == 0 else T - blk * BLK - bT
                            x = xpool.tile([128, 8 * BLK * 8], dtb,
                                           tag=f"x{d}", name=f"{name}x{d}")
                            nc.sync.dma_start(
                                x[:, :8 * bT * 8].rearrange(
                                    "p (m t) -> p m t", m=8),
                                xp_dram[d, :, :, lo * 8:(lo + bT) * 8])
                            xt[d] = (x, lo)
                    for j in range(bT):
                        s = blk * BLK + j
                        st = {}
                        # stage 1: both dirs' matmul groups
                        for d in range(2):
                            t = s if d == 0 else T - 1 - s
                            if xp_dram is not None:
                                x, lo = xt[d]
                                xap = x[:].rearrange(
                                    "p (m t b) -> p m t b", m=8, b=8
                                )[:, :, t - lo, :]
                            else:
                                xap = xp_sbuf[:].rearrange(
                                    "p (d m tok) -> p d m tok", d=2, m=8
                                )[:, d, :, t * 8:(t + 1) * 8]
                            rd = t * 32 if d == 0 else (t + 2) * 32
                            ps = pp.tile([128, 64], dtf, tag=f"ps{d}",
                                         name=f"{name}ps{d}")
                            nc.tensor.matmul(
                                ps[:].rearrange("p (m b) -> p m b", m=8),
                                ident_sb[:], xap, start=True, stop=False)
                            nc.tensor.matmul(
                                ps[:], bias_sb[:, d * 128:(d + 1) * 128],
                                bsel_sb[:], start=False, stop=False)
                            for k in range(2):
                                for m in range(8):
                                    nc.tensor.matmul(
                                        ps[:, m * 8:(m + 1) * 8],
                                        whh_sb[:, ((d * 2 + k) * 8 + m) * 128:
                                               ((d * 2 + k) * 8 + m + 1) * 128],
                                        h_sb[:, rd + d * 16 + k * 8:
                                             rd + d * 16 + (k + 1) * 8],
                                        start=False, stop=(m == 7 and k == 1))
                            st[d] = (t, ps)
                        # stage 2: ACT nonlinearities for both dirs
                        for d in range(2):
                            _, ps = st[d]
                            sg = work.tile([128, 48], dtf, tag=f"sg{d}",
                                           name=f"{name}sg{d}")
                            nc.scalar.activation(sg[:], ps[:, 0:48], AF.Sigmoid)
                            th = work.tile([128, 16], dtf, tag=f"th{d}",
                                           name=f"{name}th{d}")
                            nc.scalar.activation(th[:], ps[:, 48:64], AF.Tanh)
                            st[d] = (st[d][0], ps, sg, th)
                        # stage 3: per-dir cell update + h (f fully, then b)
                        for d in range(2):
                            t, ps, sg, th = st[d]
                            pi, po = s % 2, 1 - s % 2
                            ci = c_sb[:, (d * 2 + pi) * 16:(d * 2 + pi + 1) * 16]
                            co = c_sb[:, (d * 2 + po) * 16:(d * 2 + po + 1) * 16]
                            fc = work.tile([128, 16], dtf, tag=f"fc{d}",
                                           name=f"{name}fc{d}")
                            nc.vector.tensor_mul(fc[:], sg[:, 16:32], ci)
                            ig = work.tile([128, 16], dtf, tag=f"ig{d}",
                                           name=f"{name}ig{d}")
                            nc.vector.tensor_mul(ig[:], sg[:, 0:16], th[:])
                            nc.vector.tensor_add(co, ig[:], fc[:])
                            tcl = work.tile([128, 16], dtf, tag=f"tc{d}",
                                            name=f"{name}tc{d}")
                            nc.scalar.activation(tcl[:], co, AF.Tanh)
                            # split h write by k-half so the next step's k0
                            # matmuls can begin before the k1 half lands
                            for k in range(2):
                                nc.vector.tensor_mul(
                                    h_sb[:, (t + 1) * 32 + d * 16 + k * 8:
                                         (t + 1) * 32 + d * 16 + (k + 1) * 8],
                                    sg[:, 32 + k * 8:40 + k * 8],
                                    tcl[:, k * 8:(k + 1) * 8])

        def h_rhs(h_sb, tperchunk):
            """rhs_fn for projections reading a [128,(t,d,k,b)] h-store."""
            def fn(k, kn, c):
                d, kk = k // 2, k % 2
                off = d * 16 + kk * 8
                return h_sb[:].rearrange("p (t x) -> p t x", x=32)[
                    :, 1 + c * tperchunk:1 + (c + 1) * tperchunk,
                    off:off + 8]
            return fn

        # ================= WORD PATHWAY =================================
        with ExitStack() as wph:
            wio = wph.enter_context(tc.tile_pool(name="wio", bufs=1))
            wemb_sb = wio.tile([128, 6 * WTOK], dtb)
            nc.sync.dma_start(
                wemb_sb[:].rearrange("p (k t) -> p k t", k=6),
                wembt.rearrange("(k p) t -> p k t", p=128))
            xpw = wio.tile([128, 2 * 8 * WTOK], dtb)
            hw0 = wio.tile([128, (W + 2) * 32], dtb)
            hw1 = wio.tile([128, (W + 2) * 32], dtb)
            woutT = wio.tile([96, 32 * 128], dtb)
            ohw_sb = wio.tile([96, CTOK], dtb)
            nc.sync.dma_start(ohw_sb[:], onehotw[:])

            def evac_xpw(m, c, ps):
                d, mm = m // 8, m % 8
                nc.vector.tensor_copy(
                    xpw[:, (d * 8 + mm) * WTOK + c * 384:
                        (d * 8 + mm) * WTOK + (c + 1) * 384], ps[:])

            projection("xw0", wihw0, WORD_EMB,
                       lambda k, kn, c: wemb_sb[:, k * WTOK + c * 384:
                                                k * WTOK + (c + 1) * 384],
                       2, 384, evac_xpw)
            recurrence("rw0", W, whhw, biasw, 0, hw0, xp_sbuf=xpw)
            projection("xw1", wihw1, 512, h_rhs(hw0, 48), 2, 384, evac_xpw)
            recurrence("rw1", W, whhw, biasw, 1, hw1, xp_sbuf=xpw)

            # transpose word h1 -> stationary tiles [96, 128] per (b, d, k)
            with ExitStack() as tctx:
                tp = tctx.enter_context(
                    tc.tile_pool(name="trp", bufs=4, space=MS.PSUM))
                for b in range(BC):
                    for d in range(2):
                        for k in range(2):
                            idx = (b * 2 + d) * 2 + k
                            src = hw1[:].rearrange("p (t x) -> p t x", x=32)[
                                :, 1:W + 1, d * 16 + k * 8 + b:
                                d * 16 + k * 8 + b + 1]
                            tps = tp.tile([96, 128], dtb, tag="t", name="tps")
                            nc.tensor.transpose(tps[:], src, ident_sb[:])
                            nc.vector.tensor_copy(
                                woutT[:, idx * 128:(idx + 1) * 128], tps[:])

            # ragged expansion: wexp[kw] = word_h1^T @ onehotw (per sample)
            with ExitStack() as ectx:
                ep = ectx.enter_context(
                    tc.tile_pool(name="exp", bufs=4, space=MS.PSUM))
                for b in range(BC):
                    for d in range(2):
                        for k in range(2):
                            idx = (b * 2 + d) * 2 + k
                            kw = d * 2 + k
                            ps = ep.tile([128, 512], dtf, tag="e", name="eps")
                            nc.tensor.matmul(
                                ps[:], woutT[:, idx * 128:(idx + 1) * 128],
                                ohw_sb[:, b * 512:(b + 1) * 512],
                                start=True, stop=True)
                            nc.vector.tensor_copy(
                                wexp[:].rearrange(
                                    "p (kw t b) -> p kw t b", kw=4, b=8
                                )[:, kw, :, b], ps[:])

        # ================= CHAR PATHWAY =================================
        with ExitStack() as cphA:
            ioc = cphA.enter_context(tc.tile_pool(name="ioc", bufs=1))
            hc0 = ioc.tile([128, (L + 2) * 32], dtb)
            ohc_sb = ioc.tile([VOCAB, CTOK], dtb)
            nc.sync.dma_start(ohc_sb[:], onehotc[:])
            evp = cphA.enter_context(tc.tile_pool(name="xev", bufs=6))

            def evac_xpc(lyr):
                def fn(m, c, ps):
                    d, mm = m // 8, m % 8
                    t = evp.tile([128, 512], dtb, tag="ev", name="evt")
                    nc.vector.tensor_copy(t[:], ps[:])
                    nc.sync.dma_start(
                        xpc[lyr][d, :, mm, c * 512:(c + 1) * 512], t[:])
                return fn

            projection("xc0", w0c, VOCAB,
                       lambda k, kn, c: ohc_sb[:, c * 512:(c + 1) * 512],
                       8, 512, evac_xpc(0))
            recurrence("rc0", L, whhc, biasc, 0, hc0, xp_dram=xpc[0])
            projection("xc1", wihc1, 512, h_rhs(hc0, 64), 8, 512, evac_xpc(1))
        recurrence("rc1", L, whhc, biasc, 1, hc1, xp_dram=xpc[1])

        # ================= CLASSIFIER ===================================
        with ExitStack() as cls:
            clp = cls.enter_context(tc.tile_pool(name="clsio", bufs=1))
            h1sb = clp.tile([128, 4 * CTOK], dtb)
            logits_sb = clp.tile([TAGS, CTOK], dtf)
            b1_sb = clp.tile([128, 4], dtf)
            nc.sync.dma_start(b1_sb[:], b1d[:])
            b2_sb = clp.tile([TAGS, 1], dtf)
            nc.sync.dma_start(b2_sb[:], b2d[:])

            hc_rhs = h_rhs(hc1, 64)

            def rhs_comb(k, kn, c):
                if k < 4:
                    return hc_rhs(k, kn, c)
                return wexp[:, (k - 4) * CTOK + c * 512:
                            (k - 4) * CTOK + (c + 1) * 512]

            def evac_h1(m, c, ps):
                nc.scalar.activation(
                    h1sb[:, m * CTOK + c * 512:m * CTOK + (c + 1) * 512],
                    ps[:], AF.Relu, bias=b1_sb[:, m:m + 1])

            projection("cls1", w1d, COMB, rhs_comb, 8, 512, evac_h1)

            def evac_log(m, c, ps):
                nc.scalar.activation(
                    logits_sb[:, c * 512:(c + 1) * 512], ps[:TAGS, :],
                    AF.Identity, bias=b2_sb[:])

            with ExitStack() as lctx:
                lp = lctx.enter_context(tc.tile_pool(name="logw", bufs=1))
                lps = lctx.enter_context(
                    tc.tile_pool(name="logp", bufs=4, space=MS.PSUM))
                w2_sb = lp.tile([128, 4 * TAGS], dtb)
                nc.sync.dma_start(
                    w2_sb[:].rearrange("p (k m) -> p k m", k=4),
                    w2d.rearrange("(k p) m -> p k m", p=128))
                for c in range(8):
                    ps = lps.tile([TAGS, 512], dtf, tag="lg", name="lgps")
                    for k in range(4):
                        nc.tensor.matmul(
                            ps[:], w2_sb[:, k * TAGS:(k + 1) * TAGS],
                            h1sb[:, k * CTOK + c * 512:k * CTOK + (c + 1) * 512],
                            start=(k == 0), stop=(k == 3))
                    evac_log(0, c, ps)

            nc.sync.dma_start(out[:], logits_sb[:])

    nc.compile()
    return nc


def _get_program():
    if "nc" not in _CACHE:
        _CACHE["nc"] = _build_program()
    return _CACHE["nc"]


# ---------------------------------------------------------------- entry
def kernel(char_ids, word_embeddings, word_boundaries, char_emb_table,
           char_lstm_params, word_lstm_params, cls_W1, cls_b1, cls_W2, cls_b2,
           _trace=False, _tmpdir=None):
    from concourse.bass_utils import run_bass_kernel_spmd

    nc = _get_program()
    wts = _prep_weights(char_emb_table, char_lstm_params, word_lstm_params,
                        cls_W1, cls_b1, cls_W2, cls_b2)
    char_ids = np.asarray(char_ids)
    word_embeddings = np.asarray(word_embeddings)
    word_boundaries = np.asarray(word_boundaries)

    in_maps = []
    for c in range(NCORES):
        sl = slice(c * BC, (c + 1) * BC)
        m = _prep_core_inputs(char_ids[sl], word_embeddings[sl],
                              word_boundaries[sl])
        m.update(wts)
        in_maps.append(m)

    kw = {}
    if _trace:
        kw = dict(trace=True, tmpdir=_tmpdir)
    res = run_bass_kernel_spmd(nc, in_maps, list(range(NCORES)), **kw)

    outs = []
    for c in range(NCORES):
        lg = np.asarray(res.results[c]["logits"], np.float32)   # [15, CTOK]
        outs.append(lg.reshape(TAGS, L, BC).transpose(2, 1, 0)) # [BC, L, 15]
    full = np.concatenate(outs, axis=0)                         # [64, 512, 15]
    if _trace:
        return full, res
    return full


# revision 16
# speedup vs baseline: 1.1275x; 1.1275x over previous
"""Trainium2 Bass kernel for nn_BiLSTMDualPathway_40596030881793.

Dual-pathway BiLSTM tagger: char BiLSTM (T=512, 2 layers, bidir) + word
BiLSTM (T=96), ragged word->char expansion, 2-layer classifier.

Sharding: pure data parallelism - batch 64 split 8 ways (8 samples/core),
weights replicated. On-device compute for everything except integer index
preprocessing (one-hot encodings) and weight layout / constant folding.

Device design:
- All matmuls bf16 -> fp32 PSUM.
- LSTM recurrence in gates^T [1024, 8] layout (gates on partitions, batch on
  free). Whh is the stationary operand: 16 [128,128] tiles per direction
  step. Precomputed input projections xp and biases are injected into PSUM
  by an identity-matmul and a bias-selector matmul, so ScalarE reads gate
  preactivations straight from PSUM.
- Gate tile order [i,f,o,g]: sigmoid covers a contiguous [128,48] slab,
  tanh a [128,16] slab.
- fwd/bwd directions interleave step-by-step so their PE burst -> ACT ->
  DVE cell-update chains dovetail across engines.
- Char xp lives in DRAM (16 MB/layer bf16), streamed in 8-step blocks;
  word xp stays in SBUF.
"""
import numpy as np
import ml_dtypes

B, L, W = 64, 512, 96
VOCAB, TAGS = 64, 15
CHAR_EMB, CHAR_H = 128, 256
WORD_EMB, WORD_H = 768, 256
NCORES = 8
BC = B // NCORES
CTOK = BC * L              # 4096
WTOK = BC * W              # 768
COMB = 2 * CHAR_H + 2 * WORD_H

bf16 = ml_dtypes.bfloat16

# gate permutation: pytorch [i,f,g,o] -> ours [i,f,o,g]
_PERM = np.concatenate([np.arange(0, 512), np.arange(768, 1024), np.arange(512, 768)])

_CACHE = {}


# ---------------------------------------------------------------- host prep
def _prep_weights(char_emb_table, char_lstm_params, word_lstm_params,
                  cls_W1, cls_b1, cls_W2, cls_b2):
    def lstm_mats(params):
        out = []
        for (pf, pb) in params:
            wih_cols, whh_blocks, biasT = [], [], []
            whhT = []
            for (Wih, Whh, bih, bhh) in (pf, pb):
                Wih = np.asarray(Wih, np.float32)[_PERM]
                Whh = np.asarray(Whh, np.float32)[_PERM]
                bias = (np.asarray(bih, np.float32) + np.asarray(bhh, np.float32))[_PERM]
                wih_cols.append(Wih.T)                  # [I, 1024]
                whhT.append(Whh.T)                      # [256, 1024]
                biasT.append(bias.reshape(8, 128))
            for d in range(2):
                for k in range(2):
                    for m in range(8):
                        whh_blocks.append(
                            whhT[d][k * 128:(k + 1) * 128, m * 128:(m + 1) * 128])
            out.append((np.concatenate(wih_cols, axis=1),       # [I, 2048]
                        np.concatenate(whh_blocks, axis=1),     # [128, 4096]
                        np.stack(biasT)))                       # [2, 8, 128]
        return out

    ch = lstm_mats(char_lstm_params)
    wd = lstm_mats(word_lstm_params)
    emb = np.asarray(char_emb_table, np.float32)
    return {
        "w0c": (emb @ ch[0][0]).astype(bf16),                   # [64, 2048]
        "wihc1": ch[1][0].astype(bf16),                         # [512, 2048]
        "whhc": np.stack([ch[0][1], ch[1][1]]).astype(bf16),    # [2, 128, 4096]
        "biasc": np.stack([ch[0][2], ch[1][2]]).astype(bf16),   # [2, 2, 8, 128]
        "wihw0": wd[0][0].astype(bf16),
        "wihw1": wd[1][0].astype(bf16),
        "whhw": np.stack([wd[0][1], wd[1][1]]).astype(bf16),
        "biasw": np.stack([wd[0][2], wd[1][2]]).astype(bf16),
        "w1": np.ascontiguousarray(np.asarray(cls_W1, np.float32).T).astype(bf16),
        "b1": np.ascontiguousarray(
            np.asarray(cls_b1, np.float32).reshape(4, 128).T).astype(np.float32),
        "w2": np.ascontiguousarray(np.asarray(cls_W2, np.float32).T).astype(bf16),
        "b2": np.asarray(cls_b2, np.float32).reshape(TAGS, 1).copy(),
        "ident": np.eye(128, dtype=np.float32).astype(bf16),
        "bsel": np.repeat(np.eye(8, dtype=np.float32), 8, axis=1).astype(bf16),
    }


def _prep_core_inputs(char_ids, word_embeddings, word_boundaries):
    ids = np.asarray(char_ids)                                  # [BC, L]
    oc = np.zeros((VOCAB, BC * L), np.float32)                  # tok = t*BC+b
    oc[ids.T.reshape(-1), np.arange(BC * L)] = 1.0

    we = np.asarray(word_embeddings, np.float32)                # [BC, 96, 768]
    wembT = np.transpose(we, (2, 1, 0)).reshape(WORD_EMB, WTOK)

    wb = np.asarray(word_boundaries, np.int64)
    cs = np.cumsum(wb, axis=1)
    pos = np.arange(L)
    ow = np.zeros((BC, W, L), np.float32)
    for b in range(BC):
        wid = np.searchsorted(cs[b], pos, side="right")
        valid = wid < W
        ow[b, wid[valid], pos[valid]] = 1.0
    ow = ow.transpose(1, 0, 2).reshape(W, BC * L)               # [96, (b,t)]
    return {
        "onehotc": oc.astype(bf16),
        "wembt": np.ascontiguousarray(wembT).astype(bf16),
        "onehotw": np.ascontiguousarray(ow).astype(bf16),
    }


# ---------------------------------------------------------------- program
def _build_program():
    import concourse.bacc as bacc
    import concourse.tile as tile
    import concourse.bass as bass
    from concourse import mybir
    from contextlib import ExitStack

    AF = mybir.ActivationFunctionType
    dtb = mybir.dt.bfloat16
    dtf = mybir.dt.float32
    MS = bass.MemorySpace

    nc = bacc.Bacc("TRN2", target_bir_lowering=False, debug=False,
                   num_devices=NCORES)

    def din(name, shape, dt=dtb):
        return nc.dram_tensor(name, shape, dt, kind="ExternalInput").ap()

    onehotc = din("onehotc", [VOCAB, CTOK])
    wembt = din("wembt", [WORD_EMB, WTOK])
    onehotw = din("onehotw", [W, CTOK])
    w0c = din("w0c", [VOCAB, 2048])
    wihc1 = din("wihc1", [512, 2048])
    whhc = din("whhc", [2, 128, 4096])
    biasc = din("biasc", [2, 2, 8, 128])
    wihw0 = din("wihw0", [WORD_EMB, 2048])
    wihw1 = din("wihw1", [512, 2048])
    whhw = din("whhw", [2, 128, 4096])
    biasw = din("biasw", [2, 2, 8, 128])
    w1d = din("w1", [COMB, 512])
    b1d = din("b1", [128, 4], dtf)
    w2d = din("w2", [512, TAGS])
    b2d = din("b2", [TAGS, 1], dtf)
    identd = din("ident", [128, 128])
    bseld = din("bsel", [8, 64])
    out = nc.dram_tensor("logits", [TAGS, CTOK], dtf, kind="ExternalOutput").ap()

    xpc = [nc.dram_tensor(f"xpc{l}", [2, 128, 8, CTOK], dtb).ap()
           for l in range(2)]

    with tile.TileContext(nc) as tc, ExitStack() as top:
        const = top.enter_context(tc.tile_pool(name="const", bufs=1))
        ident_sb = const.tile([128, 128], dtb)
        nc.sync.dma_start(ident_sb[:], identd[:])
        bsel_sb = const.tile([8, 64], dtb)
        nc.sync.dma_start(bsel_sb[:], bseld[:])

        longlive = top.enter_context(tc.tile_pool(name="longlive", bufs=1))
        hc1 = longlive.tile([128, (L + 2) * 32], dtb)
        wexp = longlive.tile([128, 4 * CTOK], dtb)

        # ----------------------------------------------------- projection
        def projection(name, lhsT_dram, K, rhs_fn, nchunk, chunk, evac_fn,
                       pp=None, psbufs=1):
            """evac_fn(m, c, psum_ap); lhsT_dram [K, M*128]."""
            with ExitStack() as ctx:
                kt = (K + 127) // 128
                mt = lhsT_dram.shape[1] // 128
                pool = ctx.enter_context(tc.tile_pool(name=f"{name}w", bufs=2))
                if pp is None:
                    pp = ctx.enter_context(
                        tc.tile_pool(name=f"{name}p", bufs=4, space=MS.PSUM))
                    ptag, pbufs = "ps", None
                else:
                    ptag, pbufs = "proj", psbufs
                for m in range(mt):
                    wt = pool.tile([128, kt * 128], dtb, tag="w",
                                   name=f"{name}w{m}")
                    for k in range(kt):
                        kn = min(128, K - k * 128)
                        nc.sync.dma_start(
                            wt[:kn, k * 128:k * 128 + 128],
                            lhsT_dram[k * 128:k * 128 + kn,
                                      m * 128:(m + 1) * 128])
                    for c in range(nchunk):
                        ps = pp.tile([128, chunk], dtf, tag=ptag, bufs=pbufs,
                                     name=f"{name}ps")
                        for k in range(kt):
                            kn = min(128, K - k * 128)
                            nc.tensor.matmul(
                                ps[:], wt[:kn, k * 128:k * 128 + 128],
                                rhs_fn(k, kn, c),
                                start=(k == 0), stop=(k == kt - 1))
                        evac_fn(m, c, ps)

        # ----------------------------------------------------- recurrence
        def recurrence(name, T, whh_dram, bias_dram, layer, h_sb,
                       xp_sbuf=None, xp_dram=None):
            with ExitStack() as ctx:
                for _ in rec_gen(name, T, whh_dram, bias_dram, layer, h_sb,
                                 ctx, xp_sbuf=xp_sbuf, xp_dram=xp_dram):
                    pass

        def rec_gen(name, T, whh_dram, bias_dram, layer, h_sb, ctx,
                    xp_sbuf=None, xp_dram=None, pp=None, ps_bufs=(None, None),
                    tagpfx=None):
            if True:
                if tagpfx is None:
                    tagpfx = name
                wp = ctx.enter_context(tc.tile_pool(name=f"{name}wp", bufs=1))
                whh_sb = wp.tile([128, 4096], dtb, name=f"{name}whh")
                nc.sync.dma_start(whh_sb[:], whh_dram[layer])
                bias_sb = wp.tile([8, 2 * 128], dtb, name=f"{name}bias")
                for d in range(2):
                    nc.sync.dma_start(bias_sb[:, d * 128:(d + 1) * 128],
                                      bias_dram[layer, d])
                c_sb = wp.tile([128, 2 * 2 * 16], dtf, name=f"{name}cs")
                nc.vector.memset(c_sb[:], 0.0)
                nc.vector.memset(h_sb[:, 0:32], 0.0)
                nc.vector.memset(h_sb[:, (T + 1) * 32:(T + 2) * 32], 0.0)
                xpool = ctx.enter_context(tc.tile_pool(name=f"{name}x", bufs=3))
                work = ctx.enter_context(tc.tile_pool(name=f"{name}k", bufs=3))
                if pp is None:
                    pp = ctx.enter_context(
                        tc.tile_pool(name=f"{name}p", bufs=3, space=MS.PSUM))

                BLK = 8
                xt = {}
                for blk in range((T + BLK - 1) // BLK):
                    bT = min(BLK, T - blk * BLK)
                    if xp_dram is not None:
                        for d in range(2):
                            lo = blk * BLK if d == 0 else T - blk * BLK - bT
                            x = xpool.tile([128, 8 * BLK * 8], dtb,
                                           tag=f"x{d}", name=f"{name}x{d}")
                            nc.sync.dma_start(
                                x[:, :8 * bT * 8].rearrange(
                                    "p (m t) -> p m t", m=8),
                                xp_dram[d, :, :, lo * 8:(lo + bT) * 8])
                            xt[d] = (x, lo)
                    for j in range(bT):
                        s = blk * BLK + j
                        st = {}
                        # stage 1: both dirs' matmul groups
                        for d in range(2):
                            t = s if d == 0 else T - 1 - s
                            if xp_dram is not None:
                                x, lo = xt[d]
                                xap = x[:].rearrange(
                                    "p (m t b) -> p m t b", m=8, b=8
                                )[:, :, t - lo, :]
                            else:
                                xap = xp_sbuf[:].rearrange(
                                    "p (d m tok) -> p d m tok", d=2, m=8
                                )[:, d, :, t * 8:(t + 1) * 8]
                            rd = t * 32 if d == 0 else (t + 2) * 32
                            ps = pp.tile([128, 64], dtf, tag=f"{tagpfx}ps{d}",
                                         bufs=ps_bufs[d], name=f"{name}ps{d}")
                            nc.tensor.matmul(
                                ps[:].rearrange("p (m b) -> p m b", m=8),
                                ident_sb[:], xap, start=True, stop=False)
                            nc.tensor.matmul(
                                ps[:], bias_sb[:, d * 128:(d + 1) * 128],
                                bsel_sb[:], start=False, stop=False)
                            for k in range(2):
                                for m in range(8):
                                    nc.tensor.matmul(
                                        ps[:, m * 8:(m + 1) * 8],
                                        whh_sb[:, ((d * 2 + k) * 8 + m) * 128:
                                               ((d * 2 + k) * 8 + m + 1) * 128],
                                        h_sb[:, rd + d * 16 + k * 8:
                                             rd + d * 16 + (k + 1) * 8],
                                        start=False, stop=(m == 7 and k == 1))
                            st[d] = (t, ps)
                        # stage 2: ACT nonlinearities for both dirs
                        for d in range(2):
                            _, ps = st[d]
                            sg = work.tile([128, 48], dtf, tag=f"sg{d}",
                                           name=f"{name}sg{d}")
                            nc.scalar.activation(sg[:], ps[:, 0:48], AF.Sigmoid)
                            th = work.tile([128, 16], dtf, tag=f"th{d}",
                                           name=f"{name}th{d}")
                            nc.scalar.activation(th[:], ps[:, 48:64], AF.Tanh)
                            st[d] = (st[d][0], ps, sg, th)
                        # stage 3: per-dir cell update + h (f fully, then b)
                        for d in range(2):
                            t, ps, sg, th = st[d]
                            pi, po = s % 2, 1 - s % 2
                            ci = c_sb[:, (d * 2 + pi) * 16:(d * 2 + pi + 1) * 16]
                            co = c_sb[:, (d * 2 + po) * 16:(d * 2 + po + 1) * 16]
                            fc = work.tile([128, 16], dtf, tag=f"fc{d}",
                                           name=f"{name}fc{d}")
                            nc.vector.tensor_mul(fc[:], sg[:, 16:32], ci)
                            ig = work.tile([128, 16], dtf, tag=f"ig{d}",
                                           name=f"{name}ig{d}")
                            nc.vector.tensor_mul(ig[:], sg[:, 0:16], th[:])
                            nc.vector.tensor_add(co, ig[:], fc[:])
                            tcl = work.tile([128, 16], dtf, tag=f"tc{d}",
                                            name=f"{name}tc{d}")
                            nc.scalar.activation(tcl[:], co, AF.Tanh)
                            # split h write by k-half so the next step's k0
                            # matmuls can begin before the k1 half lands
                            for k in range(2):
                                nc.vector.tensor_mul(
                                    h_sb[:, (t + 1) * 32 + d * 16 + k * 8:
                                         (t + 1) * 32 + d * 16 + (k + 1) * 8],
                                    sg[:, 32 + k * 8:40 + k * 8],
                                    tcl[:, k * 8:(k + 1) * 8])
                        yield s

        def h_rhs(h_sb, tperchunk):
            """rhs_fn for projections reading a [128,(t,d,k,b)] h-store."""
            def fn(k, kn, c):
                d, kk = k // 2, k % 2
                off = d * 16 + kk * 8
                return h_sb[:].rearrange("p (t x) -> p t x", x=32)[
                    :, 1 + c * tperchunk:1 + (c + 1) * tperchunk,
                    off:off + 8]
            return fn

        # ================= CHAR PATHWAY =================================
        with ExitStack() as cphA:
            ioc = cphA.enter_context(tc.tile_pool(name="ioc", bufs=1))
            hc0 = ioc.tile([128, (L + 2) * 32], dtb)
            ohc_sb = ioc.tile([VOCAB, CTOK], dtb)
            nc.sync.dma_start(ohc_sb[:], onehotc[:])
            evp = cphA.enter_context(tc.tile_pool(name="xev", bufs=6))

            def evac_xpc(lyr):
                def fn(m, c, ps):
                    d, mm = m // 8, m % 8
                    t = evp.tile([128, 512], dtb, tag="ev", name="evt")
                    nc.vector.tensor_copy(t[:], ps[:])
                    nc.sync.dma_start(
                        xpc[lyr][d, :, mm, c * 512:(c + 1) * 512], t[:])
                return fn

            projection("xc0", w0c, VOCAB,
                       lambda k, kn, c: ohc_sb[:, c * 512:(c + 1) * 512],
                       8, 512, evac_xpc(0))
            recurrence("rc0", L, whhc, biasc, 0, hc0, xp_dram=xpc[0])
            projection("xc1", wihc1, 512, h_rhs(hc0, 64), 8, 512, evac_xpc(1))

        # ===== merged phase: char layer-1 recurrence + whole word pathway ==
        # The word pathway is independent of the char pathway, so its
        # projections + recurrences are interleaved into rc1's step loop to
        # soak rc1's chain-latency idle time (rc1 alone leaves every engine
        # ~50% idle).
        with ExitStack() as mph:
            wio = mph.enter_context(tc.tile_pool(name="wio", bufs=1))
            wemb_sb = wio.tile([128, 6 * WTOK], dtb)
            nc.sync.dma_start(
                wemb_sb[:].rearrange("p (k t) -> p k t", k=6),
                wembt.rearrange("(k p) t -> p k t", p=128))
            xpw = wio.tile([128, 2 * 8 * WTOK], dtb)
            hw0 = wio.tile([128, (W + 2) * 32], dtb)
            hw1 = wio.tile([128, (W + 2) * 32], dtb)
            woutT = wio.tile([96, 32 * 128], dtb)
            ohw_sb = wio.tile([96, CTOK], dtb)
            nc.sync.dma_start(ohw_sb[:], onehotw[:])
            spp = mph.enter_context(
                tc.tile_pool(name="spp", bufs=1, space=MS.PSUM))

            def evac_xpw(m, c, ps):
                d, mm = m // 8, m % 8
                nc.vector.tensor_copy(
                    xpw[:, (d * 8 + mm) * WTOK + c * 384:
                        (d * 8 + mm) * WTOK + (c + 1) * 384], ps[:])

            projection("xw0", wihw0, WORD_EMB,
                       lambda k, kn, c: wemb_sb[:, k * WTOK + c * 384:
                                                k * WTOK + (c + 1) * 384],
                       2, 384, evac_xpw, pp=spp)
            g_rc1 = rec_gen("rc1", L, whhc, biasc, 1, hc1, mph,
                            xp_dram=xpc[1], pp=spp, ps_bufs=(2, 2))
            g_w = rec_gen("rw0", W, whhw, biasw, 0, hw0, mph, xp_sbuf=xpw,
                          pp=spp, ps_bufs=(2, 1), tagpfx="w")
            wstage = 0
            for s in range(L):
                next(g_rc1)
                if wstage == 0:
                    if next(g_w, None) is None:
                        projection("xw1", wihw1, 512, h_rhs(hw0, 48),
                                   2, 384, evac_xpw, pp=spp)
                        g_w = rec_gen("rw1", W, whhw, biasw, 1, hw1, mph,
                                      xp_sbuf=xpw, pp=spp, ps_bufs=(2, 1),
                                      tagpfx="w")
                        wstage = 1
                elif wstage == 1:
                    if next(g_w, None) is None:
                        wstage = 2

            # transpose word h1 -> stationary tiles [96, 128] per (b, d, k)
            for b in range(BC):
                for d in range(2):
                    for k in range(2):
                        idx = (b * 2 + d) * 2 + k
                        src = hw1[:].rearrange("p (t x) -> p t x", x=32)[
                            :, 1:W + 1, d * 16 + k * 8 + b:
                            d * 16 + k * 8 + b + 1]
                        tps = spp.tile([96, 128], dtb, tag="proj", bufs=1,
                                       name="tps")
                        nc.tensor.transpose(tps[:], src, ident_sb[:])
                        nc.vector.tensor_copy(
                            woutT[:, idx * 128:(idx + 1) * 128], tps[:])

            # ragged expansion: wexp[kw] = word_h1^T @ onehotw (per sample)
            for b in range(BC):
                for d in range(2):
                    for k in range(2):
                        idx = (b * 2 + d) * 2 + k
                        kw = d * 2 + k
                        eps = spp.tile([128, 512], dtf, tag="proj", bufs=1,
                                       name="eps")
                        nc.tensor.matmul(
                            eps[:], woutT[:, idx * 128:(idx + 1) * 128],
                            ohw_sb[:, b * 512:(b + 1) * 512],
                            start=True, stop=True)
                        nc.vector.tensor_copy(
                            wexp[:].rearrange(
                                "p (kw t b) -> p kw t b", kw=4, b=8
                            )[:, kw, :, b], eps[:])

        # ================= CLASSIFIER ===================================
        with ExitStack() as cls:
            clp = cls.enter_context(tc.tile_pool(name="clsio", bufs=1))
            h1sb = clp.tile([128, 4 * CTOK], dtb)
            logits_sb = clp.tile([TAGS, CTOK], dtf)
            b1_sb = clp.tile([128, 4], dtf)
            nc.sync.dma_start(b1_sb[:], b1d[:])
            b2_sb = clp.tile([TAGS, 1], dtf)
            nc.sync.dma_start(b2_sb[:], b2d[:])

            hc_rhs = h_rhs(hc1, 64)

            def rhs_comb(k, kn, c):
                if k < 4:
                    return hc_rhs(k, kn, c)
                return wexp[:, (k - 4) * CTOK + c * 512:
                            (k - 4) * CTOK + (c + 1) * 512]

            def evac_h1(m, c, ps):
                nc.scalar.activation(
                    h1sb[:, m * CTOK + c * 512:m * CTOK + (c + 1) * 512],
                    ps[:], AF.Relu, bias=b1_sb[:, m:m + 1])

            projection("cls1", w1d, COMB, rhs_comb, 8, 512, evac_h1)

            def evac_log(m, c, ps):
                nc.scalar.activation(
                    logits_sb[:, c * 512:(c + 1) * 512], ps[:TAGS, :],
                    AF.Identity, bias=b2_sb[:])

            with ExitStack() as lctx:
                lp = lctx.enter_context(tc.tile_pool(name="logw", bufs=1))
                lps = lctx.enter_context(
                    tc.tile_pool(name="logp", bufs=4, space=MS.PSUM))
                w2_sb = lp.tile([128, 4 * TAGS], dtb)
                nc.sync.dma_start(
                    w2_sb[:].rearrange("p (k m) -> p k m", k=4),
                    w2d.rearrange("(k p) m -> p k m", p=128))
                for c in range(8):
                    ps = lps.tile([TAGS, 512], dtf, tag="lg", name="lgps")
                    for k in range(4):
                        nc.tensor.matmul(
                            ps[:], w2_sb[:, k * TAGS:(k + 1) * TAGS],
                            h1sb[:, k * CTOK + c * 512:k * CTOK + (c + 1) * 512],
                            start=(k == 0), stop=(k == 3))
                    evac_log(0, c, ps)

            nc.sync.dma_start(out[:], logits_sb[:])

    nc.compile()
    return nc


def _get_program():
    if "nc" not in _CACHE:
        _CACHE["nc"] = _build_program()
    return _CACHE["nc"]


# ---------------------------------------------------------------- entry
def kernel(char_ids, word_embeddings, word_boundaries, char_emb_table,
           char_lstm_params, word_lstm_params, cls_W1, cls_b1, cls_W2, cls_b2,
           _trace=False, _tmpdir=None):
    from concourse.bass_utils import run_bass_kernel_spmd

    nc = _get_program()
    wts = _prep_weights(char_emb_table, char_lstm_params, word_lstm_params,
                        cls_W1, cls_b1, cls_W2, cls_b2)
    char_ids = np.asarray(char_ids)
    word_embeddings = np.asarray(word_embeddings)
    word_boundaries = np.asarray(word_boundaries)

    in_maps = []
    for c in range(NCORES):
        sl = slice(c * BC, (c + 1) * BC)
        m = _prep_core_inputs(char_ids[sl], word_embeddings[sl],
                              word_boundaries[sl])
        m.update(wts)
        in_maps.append(m)

    kw = {}
    if _trace:
        kw = dict(trace=True, tmpdir=_tmpdir)
    res = run_bass_kernel_spmd(nc, in_maps, list(range(NCORES)), **kw)

    outs = []
    for c in range(NCORES):
        lg = np.asarray(res.results[c]["logits"], np.float32)   # [15, CTOK]
        outs.append(lg.reshape(TAGS, L, BC).transpose(2, 1, 0)) # [BC, L, 15]
    full = np.concatenate(outs, axis=0)                         # [64, 512, 15]
    if _trace:
        return full, res
    return full


# revision 17
# speedup vs baseline: 1.1695x; 1.0372x over previous
"""Trainium2 Bass kernel for nn_BiLSTMDualPathway_40596030881793.

Dual-pathway BiLSTM tagger: char BiLSTM (T=512, 2 layers, bidir) + word
BiLSTM (T=96), ragged word->char expansion, 2-layer classifier.

Sharding: pure data parallelism - batch 64 split 8 ways (8 samples/core),
weights replicated. On-device compute for everything except integer index
preprocessing (one-hot encodings) and weight layout / constant folding.

Device design:
- All matmuls bf16 -> fp32 PSUM.
- LSTM recurrence in gates^T [1024, 8] layout (gates on partitions, batch on
  free). Whh is the stationary operand: 16 [128,128] tiles per direction
  step. Precomputed input projections xp and biases are injected into PSUM
  by an identity-matmul and a bias-selector matmul, so ScalarE reads gate
  preactivations straight from PSUM.
- Gate tile order [i,f,o,g]: sigmoid covers a contiguous [128,48] slab,
  tanh a [128,16] slab.
- fwd/bwd directions interleave step-by-step so their PE burst -> ACT ->
  DVE cell-update chains dovetail across engines.
- Char xp lives in DRAM (16 MB/layer bf16), streamed in 8-step blocks;
  word xp stays in SBUF.
"""
import numpy as np
import ml_dtypes

B, L, W = 64, 512, 96
VOCAB, TAGS = 64, 15
CHAR_EMB, CHAR_H = 128, 256
WORD_EMB, WORD_H = 768, 256
NCORES = 8
BC = B // NCORES
CTOK = BC * L              # 4096
WTOK = BC * W              # 768
COMB = 2 * CHAR_H + 2 * WORD_H

bf16 = ml_dtypes.bfloat16

# gate permutation: pytorch [i,f,g,o] -> ours [i,f,o,g]
_PERM = np.concatenate([np.arange(0, 512), np.arange(768, 1024), np.arange(512, 768)])

_CACHE = {}


# ---------------------------------------------------------------- host prep
def _prep_weights(char_emb_table, char_lstm_params, word_lstm_params,
                  cls_W1, cls_b1, cls_W2, cls_b2):
    def lstm_mats(params):
        out = []
        for (pf, pb) in params:
            wih_cols, whh_blocks, biasT = [], [], []
            whhT = []
            for (Wih, Whh, bih, bhh) in (pf, pb):
                Wih = np.asarray(Wih, np.float32)[_PERM]
                Whh = np.asarray(Whh, np.float32)[_PERM]
                bias = (np.asarray(bih, np.float32) + np.asarray(bhh, np.float32))[_PERM]
                wih_cols.append(Wih.T)                  # [I, 1024]
                whhT.append(Whh.T)                      # [256, 1024]
                biasT.append(bias.reshape(8, 128))
            for d in range(2):
                for k in range(2):
                    for m in range(8):
                        whh_blocks.append(
                            whhT[d][k * 128:(k + 1) * 128, m * 128:(m + 1) * 128])
            out.append((np.concatenate(wih_cols, axis=1),       # [I, 2048]
                        np.concatenate(whh_blocks, axis=1),     # [128, 4096]
                        np.stack(biasT)))                       # [2, 8, 128]
        return out

    ch = lstm_mats(char_lstm_params)
    wd = lstm_mats(word_lstm_params)
    emb = np.asarray(char_emb_table, np.float32)
    return {
        "w0c": (emb @ ch[0][0]).astype(bf16),                   # [64, 2048]
        "wihc1": ch[1][0].astype(bf16),                         # [512, 2048]
        "whhc": np.stack([ch[0][1], ch[1][1]]).astype(bf16),    # [2, 128, 4096]
        "biasc": np.stack([ch[0][2], ch[1][2]]).astype(bf16),   # [2, 2, 8, 128]
        "wihw0": wd[0][0].astype(bf16),
        "wihw1": wd[1][0].astype(bf16),
        "whhw": np.stack([wd[0][1], wd[1][1]]).astype(bf16),
        "biasw": np.stack([wd[0][2], wd[1][2]]).astype(bf16),
        "w1": np.ascontiguousarray(np.asarray(cls_W1, np.float32).T).astype(bf16),
        "b1": np.ascontiguousarray(
            np.asarray(cls_b1, np.float32).reshape(4, 128).T).astype(np.float32),
        "w2": np.ascontiguousarray(np.asarray(cls_W2, np.float32).T).astype(bf16),
        "b2": np.asarray(cls_b2, np.float32).reshape(TAGS, 1).copy(),
        "ident": np.eye(128, dtype=np.float32).astype(bf16),
        "bsel": np.repeat(np.eye(8, dtype=np.float32), 8, axis=1).astype(bf16),
    }


def _prep_core_inputs(char_ids, word_embeddings, word_boundaries):
    ids = np.asarray(char_ids)                                  # [BC, L]
    oc = np.zeros((VOCAB, BC * L), np.float32)                  # tok = t*BC+b
    oc[ids.T.reshape(-1), np.arange(BC * L)] = 1.0

    we = np.asarray(word_embeddings, np.float32)                # [BC, 96, 768]
    wembT = np.transpose(we, (2, 1, 0)).reshape(WORD_EMB, WTOK)

    wb = np.asarray(word_boundaries, np.int64)
    cs = np.cumsum(wb, axis=1)
    pos = np.arange(L)
    ow = np.zeros((BC, W, L), np.float32)
    for b in range(BC):
        wid = np.searchsorted(cs[b], pos, side="right")
        valid = wid < W
        ow[b, wid[valid], pos[valid]] = 1.0
    ow = ow.transpose(1, 0, 2).reshape(W, BC * L)               # [96, (b,t)]
    return {
        "onehotc": oc.astype(bf16),
        "wembt": np.ascontiguousarray(wembT).astype(bf16),
        "onehotw": np.ascontiguousarray(ow).astype(bf16),
    }


# ---------------------------------------------------------------- program
def _build_program():
    import concourse.bacc as bacc
    import concourse.tile as tile
    import concourse.bass as bass
    from concourse import mybir
    from contextlib import ExitStack

    AF = mybir.ActivationFunctionType
    dtb = mybir.dt.bfloat16
    dtf = mybir.dt.float32
    MS = bass.MemorySpace

    nc = bacc.Bacc("TRN2", target_bir_lowering=False, debug=False,
                   num_devices=NCORES)

    def din(name, shape, dt=dtb):
        return nc.dram_tensor(name, shape, dt, kind="ExternalInput").ap()

    onehotc = din("onehotc", [VOCAB, CTOK])
    wembt = din("wembt", [WORD_EMB, WTOK])
    onehotw = din("onehotw", [W, CTOK])
    w0c = din("w0c", [VOCAB, 2048])
    wihc1 = din("wihc1", [512, 2048])
    whhc = din("whhc", [2, 128, 4096])
    biasc = din("biasc", [2, 2, 8, 128])
    wihw0 = din("wihw0", [WORD_EMB, 2048])
    wihw1 = din("wihw1", [512, 2048])
    whhw = din("whhw", [2, 128, 4096])
    biasw = din("biasw", [2, 2, 8, 128])
    w1d = din("w1", [COMB, 512])
    b1d = din("b1", [128, 4], dtf)
    w2d = din("w2", [512, TAGS])
    b2d = din("b2", [TAGS, 1], dtf)
    identd = din("ident", [128, 128])
    bseld = din("bsel", [8, 64])
    out = nc.dram_tensor("logits", [TAGS, CTOK], dtf, kind="ExternalOutput").ap()

    xpc = [nc.dram_tensor(f"xpc{l}", [2, 128, 8, CTOK], dtb).ap()
           for l in range(2)]

    with tile.TileContext(nc) as tc, ExitStack() as top:
        const = top.enter_context(tc.tile_pool(name="const", bufs=1))
        ident_sb = const.tile([128, 128], dtb)
        nc.sync.dma_start(ident_sb[:], identd[:])
        bsel_sb = const.tile([8, 64], dtb)
        nc.sync.dma_start(bsel_sb[:], bseld[:])

        longlive = top.enter_context(tc.tile_pool(name="longlive", bufs=1))
        hc1 = longlive.tile([128, (L + 2) * 32], dtb)
        wexp = longlive.tile([128, 4 * CTOK], dtb)

        # ----------------------------------------------------- projection
        def projection(name, lhsT_dram, K, rhs_fn, nchunk, chunk, evac_fn,
                       pp=None, psbufs=1):
            """evac_fn(m, c, psum_ap); lhsT_dram [K, M*128]."""
            with ExitStack() as ctx:
                kt = (K + 127) // 128
                mt = lhsT_dram.shape[1] // 128
                pool = ctx.enter_context(tc.tile_pool(name=f"{name}w", bufs=2))
                if pp is None:
                    pp = ctx.enter_context(
                        tc.tile_pool(name=f"{name}p", bufs=4, space=MS.PSUM))
                    ptag, pbufs = "ps", None
                else:
                    ptag, pbufs = "proj", psbufs
                for m in range(mt):
                    wt = pool.tile([128, kt * 128], dtb, tag="w",
                                   name=f"{name}w{m}")
                    for k in range(kt):
                        kn = min(128, K - k * 128)
                        nc.sync.dma_start(
                            wt[:kn, k * 128:k * 128 + 128],
                            lhsT_dram[k * 128:k * 128 + kn,
                                      m * 128:(m + 1) * 128])
                    for c in range(nchunk):
                        ps = pp.tile([128, chunk], dtf, tag=ptag, bufs=pbufs,
                                     name=f"{name}ps")
                        for k in range(kt):
                            kn = min(128, K - k * 128)
                            nc.tensor.matmul(
                                ps[:], wt[:kn, k * 128:k * 128 + 128],
                                rhs_fn(k, kn, c),
                                start=(k == 0), stop=(k == kt - 1))
                        evac_fn(m, c, ps)

        # ----------------------------------------------------- recurrence
        def recurrence(name, T, whh_dram, bias_dram, layer, h_sb,
                       xp_sbuf=None, xp_dram=None):
            with ExitStack() as ctx:
                for _ in rec_gen(name, T, whh_dram, bias_dram, layer, h_sb,
                                 ctx, xp_sbuf=xp_sbuf, xp_dram=xp_dram):
                    pass

        def rec_gen(name, T, whh_dram, bias_dram, layer, h_sb, ctx,
                    xp_sbuf=None, xp_dram=None, pp=None, ps_bufs=(None, None),
                    tagpfx=None):
            if True:
                if tagpfx is None:
                    tagpfx = name
                wp = ctx.enter_context(tc.tile_pool(name=f"{name}wp", bufs=1))
                whh_sb = wp.tile([128, 4096], dtb, name=f"{name}whh")
                nc.sync.dma_start(whh_sb[:], whh_dram[layer])
                bias_sb = wp.tile([8, 2 * 128], dtb, name=f"{name}bias")
                for d in range(2):
                    nc.sync.dma_start(bias_sb[:, d * 128:(d + 1) * 128],
                                      bias_dram[layer, d])
                c_sb = wp.tile([128, 2 * 2 * 16], dtf, name=f"{name}cs")
                nc.vector.memset(c_sb[:], 0.0)
                nc.vector.memset(h_sb[:, 0:32], 0.0)
                nc.vector.memset(h_sb[:, (T + 1) * 32:(T + 2) * 32], 0.0)
                xpool = ctx.enter_context(tc.tile_pool(name=f"{name}x", bufs=3))
                work = ctx.enter_context(tc.tile_pool(name=f"{name}k", bufs=3))
                if pp is None:
                    pp = ctx.enter_context(
                        tc.tile_pool(name=f"{name}p", bufs=3, space=MS.PSUM))

                BLK = 8
                xt = {}
                for blk in range((T + BLK - 1) // BLK):
                    bT = min(BLK, T - blk * BLK)
                    if xp_dram is not None:
                        for d in range(2):
                            lo = blk * BLK if d == 0 else T - blk * BLK - bT
                            x = xpool.tile([128, 8 * BLK * 8], dtb,
                                           tag=f"x{d}", name=f"{name}x{d}")
                            nc.sync.dma_start(
                                x[:, :8 * bT * 8].rearrange(
                                    "p (m t) -> p m t", m=8),
                                xp_dram[d, :, :, lo * 8:(lo + bT) * 8])
                            xt[d] = (x, lo)
                    for j in range(bT):
                        s = blk * BLK + j
                        st = {}
                        # stage 1: both dirs' matmul groups
                        for d in range(2):
                            t = s if d == 0 else T - 1 - s
                            if xp_dram is not None:
                                x, lo = xt[d]
                                xap = x[:].rearrange(
                                    "p (m t b) -> p m t b", m=8, b=8
                                )[:, :, t - lo, :]
                            else:
                                xap = xp_sbuf[:].rearrange(
                                    "p (d m tok) -> p d m tok", d=2, m=8
                                )[:, d, :, t * 8:(t + 1) * 8]
                            rd = t * 32 if d == 0 else (t + 2) * 32
                            ps = pp.tile([128, 64], dtf, tag=f"{tagpfx}ps{d}",
                                         bufs=ps_bufs[d], name=f"{name}ps{d}")
                            nc.tensor.matmul(
                                ps[:].rearrange("p (m b) -> p m b", m=8),
                                ident_sb[:], xap, start=True, stop=False)
                            nc.tensor.matmul(
                                ps[:], bias_sb[:, d * 128:(d + 1) * 128],
                                bsel_sb[:], start=False, stop=False)
                            for k in range(2):
                                for m in range(8):
                                    nc.tensor.matmul(
                                        ps[:, m * 8:(m + 1) * 8],
                                        whh_sb[:, ((d * 2 + k) * 8 + m) * 128:
                                               ((d * 2 + k) * 8 + m + 1) * 128],
                                        h_sb[:, rd + d * 16 + k * 8:
                                             rd + d * 16 + (k + 1) * 8],
                                        start=False, stop=(m == 7 and k == 1))
                            st[d] = (t, ps)
                        # stage 2: ACT nonlinearities for both dirs
                        for d in range(2):
                            _, ps = st[d]
                            sg = work.tile([128, 48], dtf, tag=f"sg{d}",
                                           name=f"{name}sg{d}")
                            nc.scalar.activation(sg[:], ps[:, 0:48], AF.Sigmoid)
                            th = work.tile([128, 16], dtf, tag=f"th{d}",
                                           name=f"{name}th{d}")
                            nc.scalar.activation(th[:], ps[:, 48:64], AF.Tanh)
                            st[d] = (st[d][0], ps, sg, th)
                        # stage 3: per-dir cell update + h (f fully, then b)
                        for d in range(2):
                            t, ps, sg, th = st[d]
                            pi, po = s % 2, 1 - s % 2
                            ci = c_sb[:, (d * 2 + pi) * 16:(d * 2 + pi + 1) * 16]
                            co = c_sb[:, (d * 2 + po) * 16:(d * 2 + po + 1) * 16]
                            fc = work.tile([128, 16], dtf, tag=f"fc{d}",
                                           name=f"{name}fc{d}")
                            nc.vector.tensor_mul(fc[:], sg[:, 16:32], ci)
                            ig = work.tile([128, 16], dtf, tag=f"ig{d}",
                                           name=f"{name}ig{d}")
                            nc.vector.tensor_mul(ig[:], sg[:, 0:16], th[:])
                            nc.vector.tensor_add(co, ig[:], fc[:])
                            tcl = work.tile([128, 16], dtf, tag=f"tc{d}",
                                            name=f"{name}tc{d}")
                            nc.scalar.activation(tcl[:], co, AF.Tanh)
                            # split h write by k-half so the next step's k0
                            # matmuls can begin before the k1 half lands
                            for k in range(2):
                                nc.vector.tensor_mul(
                                    h_sb[:, (t + 1) * 32 + d * 16 + k * 8:
                                         (t + 1) * 32 + d * 16 + (k + 1) * 8],
                                    sg[:, 32 + k * 8:40 + k * 8],
                                    tcl[:, k * 8:(k + 1) * 8])
                        yield s

        def h_rhs(h_sb, tperchunk):
            """rhs_fn for projections reading a [128,(t,d,k,b)] h-store."""
            def fn(k, kn, c):
                d, kk = k // 2, k % 2
                off = d * 16 + kk * 8
                return h_sb[:].rearrange("p (t x) -> p t x", x=32)[
                    :, 1 + c * tperchunk:1 + (c + 1) * tperchunk,
                    off:off + 8]
            return fn

        # ================= CHAR PATHWAY =================================
        with ExitStack() as cphA:
            ioc = cphA.enter_context(tc.tile_pool(name="ioc", bufs=1))
            hc0 = ioc.tile([128, (L + 2) * 32], dtb)
            ohc_sb = ioc.tile([VOCAB, CTOK], dtb)
            nc.sync.dma_start(ohc_sb[:], onehotc[:])
            evp = cphA.enter_context(tc.tile_pool(name="xev", bufs=6))

            def evac_xpc(lyr):
                def fn(m, c, ps):
                    d, mm = m // 8, m % 8
                    t = evp.tile([128, 512], dtb, tag="ev", name="evt")
                    nc.vector.tensor_copy(t[:], ps[:])
                    nc.sync.dma_start(
                        xpc[lyr][d, :, mm, c * 512:(c + 1) * 512], t[:])
                return fn

            projection("xc0", w0c, VOCAB,
                       lambda k, kn, c: ohc_sb[:, c * 512:(c + 1) * 512],
                       8, 512, evac_xpc(0))
            recurrence("rc0", L, whhc, biasc, 0, hc0, xp_dram=xpc[0])
            projection("xc1", wihc1, 512, h_rhs(hc0, 64), 8, 512, evac_xpc(1))

        # ===== merged phase: char layer-1 recurrence + whole word pathway ==
        # The word pathway is independent of the char pathway, so its
        # projections + recurrences are interleaved into rc1's step loop to
        # soak rc1's chain-latency idle time (rc1 alone leaves every engine
        # ~50% idle).
        with ExitStack() as mph:
            wio = mph.enter_context(tc.tile_pool(name="wio", bufs=1))
            wemb_sb = wio.tile([128, 6 * WTOK], dtb)
            nc.sync.dma_start(
                wemb_sb[:].rearrange("p (k t) -> p k t", k=6),
                wembt.rearrange("(k p) t -> p k t", p=128))
            xpw = wio.tile([128, 2 * 8 * WTOK], dtb)
            hw0 = wio.tile([128, (W + 2) * 32], dtb)
            hw1 = wio.tile([128, (W + 2) * 32], dtb)
            woutT = wio.tile([96, 32 * 128], dtb)
            ohw_sb = wio.tile([96, CTOK], dtb)
            nc.sync.dma_start(ohw_sb[:], onehotw[:])
            spp = mph.enter_context(
                tc.tile_pool(name="spp", bufs=1, space=MS.PSUM))

            def evac_xpw(m, c, ps):
                d, mm = m // 8, m % 8
                nc.vector.tensor_copy(
                    xpw[:, (d * 8 + mm) * WTOK + c * 384:
                        (d * 8 + mm) * WTOK + (c + 1) * 384], ps[:])

            projection("xw0", wihw0, WORD_EMB,
                       lambda k, kn, c: wemb_sb[:, k * WTOK + c * 384:
                                                k * WTOK + (c + 1) * 384],
                       2, 384, evac_xpw, pp=spp)
            g_rc1 = rec_gen("rc1", L, whhc, biasc, 1, hc1, mph,
                            xp_dram=xpc[1], pp=spp, ps_bufs=(2, 2))
            g_w = rec_gen("rw0", W, whhw, biasw, 0, hw0, mph, xp_sbuf=xpw,
                          pp=spp, ps_bufs=(2, 1), tagpfx="w")
            # pace the word pathway across rc1's whole span (Bresenham) so
            # rc1 has overlap work for all 512 steps, not just the first 200
            HALF = 250
            wstage, wtaken = 0, 0
            for s in range(L):
                next(g_rc1)
                if wstage == 0:
                    if wtaken * HALF < W * min(s + 1, HALF):
                        wtaken += 1
                        if next(g_w, None) is None:
                            wstage = 3
                    if wtaken >= W or s >= HALF:
                        if wstage == 0:
                            for _ in g_w:   # drain any remainder
                                pass
                        projection("xw1", wihw1, 512, h_rhs(hw0, 48),
                                   2, 384, evac_xpw, pp=spp)
                        g_w = rec_gen("rw1", W, whhw, biasw, 1, hw1, mph,
                                      xp_sbuf=xpw, pp=spp, ps_bufs=(2, 1),
                                      tagpfx="w")
                        wstage, wtaken = 1, 0
                elif wstage == 1:
                    if wtaken * HALF < W * min(s + 1 - HALF, HALF):
                        wtaken += 1
                        if next(g_w, None) is None:
                            wstage = 2
            for _ in g_w:
                pass

            # transpose word h1 -> stationary tiles [96, 128] per (b, d, k)
            for b in range(BC):
                for d in range(2):
                    for k in range(2):
                        idx = (b * 2 + d) * 2 + k
                        src = hw1[:].rearrange("p (t x) -> p t x", x=32)[
                            :, 1:W + 1, d * 16 + k * 8 + b:
                            d * 16 + k * 8 + b + 1]
                        tps = spp.tile([96, 128], dtb, tag="proj", bufs=1,
                                       name="tps")
                        nc.tensor.transpose(tps[:], src, ident_sb[:])
                        nc.vector.tensor_copy(
                            woutT[:, idx * 128:(idx + 1) * 128], tps[:])

            # ragged expansion: wexp[kw] = word_h1^T @ onehotw (per sample)
            for b in range(BC):
                for d in range(2):
                    for k in range(2):
                        idx = (b * 2 + d) * 2 + k
                        kw = d * 2 + k
                        eps = spp.tile([128, 512], dtf, tag="proj", bufs=1,
                                       name="eps")
                        nc.tensor.matmul(
                            eps[:], woutT[:, idx * 128:(idx + 1) * 128],
                            ohw_sb[:, b * 512:(b + 1) * 512],
                            start=True, stop=True)
                        nc.vector.tensor_copy(
                            wexp[:].rearrange(
                                "p (kw t b) -> p kw t b", kw=4, b=8
                            )[:, kw, :, b], eps[:])

        # ================= CLASSIFIER ===================================
        with ExitStack() as cls:
            clp = cls.enter_context(tc.tile_pool(name="clsio", bufs=1))
            h1sb = clp.tile([128, 4 * CTOK], dtb)
            logits_sb = clp.tile([TAGS, CTOK], dtf)
            b1_sb = clp.tile([128, 4], dtf)
            nc.sync.dma_start(b1_sb[:], b1d[:])
            b2_sb = clp.tile([TAGS, 1], dtf)
            nc.sync.dma_start(b2_sb[:], b2d[:])

            hc_rhs = h_rhs(hc1, 64)

            def rhs_comb(k, kn, c):
                if k < 4:
                    return hc_rhs(k, kn, c)
                return wexp[:, (k - 4) * CTOK + c * 512:
                            (k - 4) * CTOK + (c + 1) * 512]

            def evac_h1(m, c, ps):
                nc.scalar.activation(
                    h1sb[:, m * CTOK + c * 512:m * CTOK + (c + 1) * 512],
                    ps[:], AF.Relu, bias=b1_sb[:, m:m + 1])

            projection("cls1", w1d, COMB, rhs_comb, 8, 512, evac_h1)

            def evac_log(m, c, ps):
                nc.scalar.activation(
                    logits_sb[:, c * 512:(c + 1) * 512], ps[:TAGS, :],
                    AF.Identity, bias=b2_sb[:])

            with ExitStack() as lctx:
                lp = lctx.enter_context(tc.tile_pool(name="logw", bufs=1))
                lps = lctx.enter_context(
                    tc.tile_pool(name="logp", bufs=4, space=MS.PSUM))
                w2_sb = lp.tile([128, 4 * TAGS], dtb)
                nc.sync.dma_start(
                    w2_sb[:].rearrange("p (k m) -> p k m", k=4),
                    w2d.rearrange("(k p) m -> p k m", p=128))
                for c in range(8):
                    ps = lps.tile([TAGS, 512], dtf, tag="lg", name="lgps")
                    for k in range(4):
                        nc.tensor.matmul(
                            ps[:], w2_sb[:, k * TAGS:(k + 1) * TAGS],
                            h1sb[:, k * CTOK + c * 512:k * CTOK + (c + 1) * 512],
                            start=(k == 0), stop=(k == 3))
                    evac_log(0, c, ps)

            nc.sync.dma_start(out[:], logits_sb[:])

    nc.compile()
    return nc


def _get_program():
    if "nc" not in _CACHE:
        _CACHE["nc"] = _build_program()
    return _CACHE["nc"]


# ---------------------------------------------------------------- entry
def kernel(char_ids, word_embeddings, word_boundaries, char_emb_table,
           char_lstm_params, word_lstm_params, cls_W1, cls_b1, cls_W2, cls_b2,
           _trace=False, _tmpdir=None):
    from concourse.bass_utils import run_bass_kernel_spmd

    nc = _get_program()
    wts = _prep_weights(char_emb_table, char_lstm_params, word_lstm_params,
                        cls_W1, cls_b1, cls_W2, cls_b2)
    char_ids = np.asarray(char_ids)
    word_embeddings = np.asarray(word_embeddings)
    word_boundaries = np.asarray(word_boundaries)

    in_maps = []
    for c in range(NCORES):
        sl = slice(c * BC, (c + 1) * BC)
        m = _prep_core_inputs(char_ids[sl], word_embeddings[sl],
                              word_boundaries[sl])
        m.update(wts)
        in_maps.append(m)

    kw = {}
    if _trace:
        kw = dict(trace=True, tmpdir=_tmpdir)
    res = run_bass_kernel_spmd(nc, in_maps, list(range(NCORES)), **kw)

    outs = []
    for c in range(NCORES):
        lg = np.asarray(res.results[c]["logits"], np.float32)   # [15, CTOK]
        outs.append(lg.reshape(TAGS, L, BC).transpose(2, 1, 0)) # [BC, L, 15]
    full = np.concatenate(outs, axis=0)                         # [64, 512, 15]
    if _trace:
        return full, res
    return full
